# revision 6
# baseline (speedup 1.0000x reference)
"""Trainium2 Bass kernel for nn_Decoder_45363444580423.

Math (per batch row b):
  enc_proj = enc[b] @ W_ref.T                              # [NN, H]
  LSTM chain over t (input = prev hidden, attention output unused by the
  recurrence, so the chain decouples from attention):
    gates = hid @ (W_ih+W_hh).T + (b_ih+b_hh); i,f,g,o = split(gates)
    cell = sig(f)*cell + sig(i)*tanh(g); hid = sig(o)*tanh(cell)
    q[t] = hid @ W_q.T
  logits[b, t, n] = sum_h v[h] * tanh(enc_proj[n, h] + q[t, h])

Strategy (pure data parallel over B across 8 cores, B_loc = 32):
  Phase 1: run the whole LSTM chain (vectorized over the 32 local batch
    rows, layout [h-chunk partitions, b free]), store qT per step to a
    DRAM scratch laid out [b, hc, 128, T] so phase 2 loads are contiguous.
  Phase 2 (hardware For_i loop over b): transpose enc[b] with PE,
    matmul to enc_projT [k, n]; then for each n: DVE tensor_scalar
    broadcast-add S = qb + enc_projT[:, n], batched in-place ACT tanh,
    and PE matmuls [K=128, M=128(t-block), N=1] with rhs = v chunk that
    accumulate the h-chunks into psum columns; DVE copies psum into the
    [t, n] logits tile, DMA out.

Engines: ACT is the bottleneck (one tanh per element of [B,T,NN,H]);
DVE (broadcast adds) and PE (v-reduction) run underneath it.
"""
import os

os.environ.setdefault("JAX_PLATFORMS", "axon")

from contextlib import ExitStack

import numpy as np

import concourse.bass as bass
import concourse.tile as tile
from concourse import bacc, mybir
from concourse.bass_utils import run_bass_kernel_spmd

F32 = mybir.dt.float32
N_CORES = 8
B_FULL, T_FULL, NN_FULL, H = 256, 512, 512, 256
HC = H // 128  # h chunks on partitions


def build(b_loc=32, t_steps=512, nn=512, nb=8, t_unroll=8, num_devices=N_CORES,
          ts_engine="vector"):
    """Emit the SPMD program for one core; returns compiled Bacc."""
    assert nn % 128 == 0 or nn < 128
    tbsz = min(128, t_steps)
    assert t_steps % tbsz == 0
    tb_cnt = t_steps // tbsz
    assert nn % nb == 0
    n_groups = nn // nb

    nc = bacc.Bacc("TRN2", target_bir_lowering=False, debug=False,
                   num_devices=num_devices)

    enc_d = nc.dram_tensor("enc", [b_loc, nn, H], F32, kind="ExternalInput")
    wsumT_d = nc.dram_tensor("wsumT", [H, 4 * H], F32, kind="ExternalInput")
    wqT_d = nc.dram_tensor("wqT", [H, H], F32, kind="ExternalInput")
    wrefT_d = nc.dram_tensor("wrefT", [H, H], F32, kind="ExternalInput")
    bsum_d = nc.dram_tensor("bsum", [8, 128], F32, kind="ExternalInput")
    v_d = nc.dram_tensor("v2", [HC, 128], F32, kind="ExternalInput")
    ident_d = nc.dram_tensor("ident", [128, 128], F32, kind="ExternalInput")
    out_d = nc.dram_tensor("logits", [b_loc, t_steps, nn], F32,
                           kind="ExternalOutput")

    with tile.TileContext(nc) as tc, ExitStack() as ctx:
        const = ctx.enter_context(tc.tile_pool(name="const", bufs=1))
        dram = ctx.enter_context(tc.tile_pool(name="dram", bufs=1, space="DRAM"))

        # ---- constants ----
        wsumT = [const.tile([128, 4 * H], F32, tag=f"wsumT{c}", name=f"wsumT{c}") for c in range(HC)]
        for c in range(HC):
            nc.sync.dma_start(wsumT[c][:], wsumT_d[c * 128:(c + 1) * 128, :])
        wqT = [const.tile([128, H], F32, tag=f"wqT{c}", name=f"wqT{c}") for c in range(HC)]
        for c in range(HC):
            nc.sync.dma_start(wqT[c][:], wqT_d[c * 128:(c + 1) * 128, :])
        wrefT = [const.tile([128, H], F32, tag=f"wrefT{c}", name=f"wrefT{c}") for c in range(HC)]
        for c in range(HC):
            nc.sync.dma_start(wrefT[c][:], wrefT_d[c * 128:(c + 1) * 128, :])
        bsum_sb = const.tile([128, 8], F32, tag="bsum")
        nc.sync.dma_start(bsum_sb[:], bsum_d.ap().transpose([1, 0]))
        v_sb = const.tile([128, HC], F32, tag="v")
        nc.sync.dma_start(v_sb[:], v_d.ap().transpose([1, 0]))
        ident_sb = const.tile([128, 128], F32, tag="ident")
        nc.sync.dma_start(ident_sb[:], ident_d[:, :])

        q_dram = dram.tile([b_loc, HC, 128, t_steps], F32, tag="q_scratch")

        # ---- phase 1: LSTM chain ----
        state = ctx.enter_context(tc.tile_pool(name="state", bufs=1))
        hidT = [state.tile([128, b_loc], F32, tag=f"hidT{c}", name=f"hidT{c}") for c in range(HC)]
        cellT = [state.tile([128, b_loc], F32, tag=f"cellT{c}", name=f"cellT{c}") for c in range(HC)]
        for c in range(HC):
            nc.vector.memset(hidT[c][:], 0.0)
            nc.vector.memset(cellT[c][:], 0.0)

        with tc.tile_pool(name="psg", bufs=2, space="PSUM") as psg_pool, \
             tc.tile_pool(name="psq", bufs=2, space="PSUM") as psq_pool, \
             tc.tile_pool(name="ph1sb", bufs=2) as ph1:

            def lstm_step(t):
                ps_g = psg_pool.tile([128, 8 * b_loc], F32, tag="psg")
                for jc in range(8):
                    for c in range(HC):
                        nc.tensor.matmul(
                            ps_g[:, jc * b_loc:(jc + 1) * b_loc],
                            wsumT[c][:, jc * 128:(jc + 1) * 128],
                            hidT[c][:],
                            start=(c == 0), stop=(c == HC - 1))
                act = ph1.tile([128, 8 * b_loc], F32, tag="act")
                for jc in range(8):
                    func = (mybir.ActivationFunctionType.Tanh if jc in (4, 5)
                            else mybir.ActivationFunctionType.Sigmoid)
                    nc.scalar.activation(
                        act[:, jc * b_loc:(jc + 1) * b_loc],
                        ps_g[:, jc * b_loc:(jc + 1) * b_loc],
                        func, bias=bsum_sb[:, jc:jc + 1])

                def gate(kind, c):  # kind: 0=i 1=f 2=g 3=o
                    jc = 2 * kind + c
                    return act[:, jc * b_loc:(jc + 1) * b_loc]

                for c in range(HC):
                    t1 = ph1.tile([128, b_loc], F32, tag=f"t1_{c}")
                    nc.vector.tensor_mul(t1[:], gate(1, c), cellT[c][:])
                    t2 = ph1.tile([128, b_loc], F32, tag=f"t2_{c}")
                    nc.vector.tensor_mul(t2[:], gate(0, c), gate(2, c))
                    nc.vector.tensor_add(cellT[c][:], t1[:], t2[:])
                    tcc = ph1.tile([128, b_loc], F32, tag=f"tcc_{c}")
                    nc.scalar.activation(tcc[:], cellT[c][:],
                                         mybir.ActivationFunctionType.Tanh)
                    nc.vector.tensor_mul(hidT[c][:], gate(3, c), tcc[:])

                ps_q = psq_pool.tile([128, HC * b_loc], F32, tag="psq")
                for cq in range(HC):
                    for c in range(HC):
                        nc.tensor.matmul(
                            ps_q[:, cq * b_loc:(cq + 1) * b_loc],
                            wqT[c][:, cq * 128:(cq + 1) * 128],
                            hidT[c][:],
                            start=(c == 0), stop=(c == HC - 1))
                qsb = ph1.tile([128, HC * b_loc], F32, tag="qsb")
                nc.vector.tensor_copy(qsb[:], ps_q[:])
                for c in range(HC):
                    nc.sync.dma_start(
                        q_dram[:, c, :, bass.ds(t, 1)].transpose([1, 0, 2]),
                        qsb[:, c * b_loc:(c + 1) * b_loc])

            tc.For_i_unrolled(0, t_steps, 1, lstm_step, max_unroll=t_unroll)

        # ---- phase 2: attention sweep, For_i over b ----
        ts_eng = getattr(nc, ts_engine)
        with tc.tile_pool(name="eraw", bufs=2) as eraw_pool, \
             tc.tile_pool(name="pstr", bufs=2, space="PSUM") as pstr_pool, \
             tc.tile_pool(name="psep", bufs=2, space="PSUM") as psep_pool, \
             tc.tile_pool(name="psmm", bufs=2, space="PSUM") as psmm_pool, \
             tc.tile_pool(name="ph2big", bufs=2) as big, \
             tc.tile_pool(name="stgp", bufs=4) as stgp, \
             tc.tile_pool(name="lgp", bufs=tb_cnt) as lgp:

            with tc.For_i(0, b_loc) as b:
                # prologue: encT = enc[b].T (PE transpose), enc_projT = WrefT-mm
                encT = [big.tile([128, nn], F32, tag="encT", name="encT") for _ in range(HC)]
                for nbk in range(max(1, nn // 128)):
                    nsz = min(128, nn)
                    enc_raw = eraw_pool.tile([nsz, H], F32, tag="eraw")
                    nc.sync.dma_start(
                        enc_raw[:],
                        enc_d[bass.ds(b, 1), nbk * nsz:(nbk + 1) * nsz, :])
                    for c in range(HC):
                        tr = pstr_pool.tile([128, nsz], F32, tag="pstr")
                        nc.tensor.transpose(tr[:], enc_raw[:, c * 128:(c + 1) * 128],
                                            ident_sb[:nsz, :nsz])
                        nc.vector.tensor_copy(
                            encT[c][:, nbk * nsz:(nbk + 1) * nsz], tr[:])
                eprojT = [big.tile([128, nn], F32, tag="eprojT", name="eprojT") for _ in range(HC)]
                for kc in range(HC):
                    ps_ep = psep_pool.tile([128, nn], F32, tag="psep")
                    for c in range(HC):
                        nc.tensor.matmul(ps_ep[:], wrefT[c][:, kc * 128:(kc + 1) * 128],
                                         encT[c][:], start=(c == 0), stop=(c == HC - 1))
                    nc.vector.tensor_copy(eprojT[kc][:], ps_ep[:])
                qb = [big.tile([128, t_steps], F32, tag="qb", name="qb") for _ in range(HC)]
                for c in range(HC):
                    nc.sync.dma_start(qb[c][:], q_dram[bass.ds(b, 1), c, :, :])

                lgsb = [lgp.tile([tbsz, nn], F32, tag="lgsb", name="lgsb") for _ in range(tb_cnt)]

                for g in range(n_groups):
                    stg = [stgp.tile([128, nb * t_steps], F32, tag="stg", name="stg")
                           for _ in range(HC)]
                    for i in range(nb):
                        n = g * nb + i
                        for c in range(HC):
                            ts_eng.tensor_scalar_add(
                                stg[c][:, i * t_steps:(i + 1) * t_steps],
                                qb[c][:], eprojT[c][:, n:n + 1])
                    for c in range(HC):
                        nc.scalar.activation(stg[c][:], stg[c][:],
                                             mybir.ActivationFunctionType.Tanh)
                    ps_mm = psmm_pool.tile([tbsz, tb_cnt * nb], F32, tag="psmm")
                    for i in range(nb):
                        for tb in range(tb_cnt):
                            for c in range(HC):
                                nc.tensor.matmul(
                                    ps_mm[:, tb * nb + i:tb * nb + i + 1],
                                    stg[c][:, i * t_steps + tb * tbsz:
                                           i * t_steps + tb * tbsz + tbsz],
                                    v_sb[:, c:c + 1],
                                    start=(c == 0), stop=(c == HC - 1))
                    for tb in range(tb_cnt):
                        nc.vector.tensor_copy(
                            lgsb[tb][:, g * nb:(g + 1) * nb],
                            ps_mm[:, tb * nb:(tb + 1) * nb])

                for tb in range(tb_cnt):
                    nc.sync.dma_start(
                        out_d[bass.ds(b, 1), tb * tbsz:(tb + 1) * tbsz, :],
                        lgsb[tb][:])

    nc.compile()
    return nc


_NC_CACHE = {}


def kernel(**inputs):
    return _run(inputs)


def _run(inputs, trace=False, build_kwargs=None):
    enc = np.ascontiguousarray(np.asarray(inputs["encoder_outputs"], np.float32))
    W_ih = np.asarray(inputs["W_ih"], np.float32)
    W_hh = np.asarray(inputs["W_hh"], np.float32)
    b_ih = np.asarray(inputs["b_ih"], np.float32)
    b_hh = np.asarray(inputs["b_hh"], np.float32)
    W_ref = np.asarray(inputs["W_ref"], np.float32)
    W_q = np.asarray(inputs["W_q"], np.float32)
    v = np.asarray(inputs["v"], np.float32)

    wsumT = np.ascontiguousarray((W_ih + W_hh).T)
    wqT = np.ascontiguousarray(W_q.T)
    wrefT = np.ascontiguousarray(W_ref.T)
    bsum = np.ascontiguousarray((b_ih + b_hh).reshape(8, 128))
    v2 = np.ascontiguousarray(v.reshape(HC, 128))
    ident = np.eye(128, dtype=np.float32)

    bk = tuple(sorted((build_kwargs or {}).items()))
    if bk not in _NC_CACHE:
        _NC_CACHE[bk] = build(**dict(bk))
    nc = _NC_CACHE[bk]
    b_loc = B_FULL // N_CORES
    in_maps = []
    for core in range(N_CORES):
        in_maps.append({
            "enc": np.ascontiguousarray(enc[core * b_loc:(core + 1) * b_loc]),
            "wsumT": wsumT, "wqT": wqT, "wrefT": wrefT,
            "bsum": bsum, "v2": v2, "ident": ident,
        })
    res = run_bass_kernel_spmd(nc, in_maps, core_ids=list(range(N_CORES)),
                               trace=trace)
    out = np.concatenate([res.results[c]["logits"] for c in range(N_CORES)],
                         axis=0)
    if trace:
        return out, res
    return out


if __name__ == "__main__":
    import reference  # only for a manual smoke run; not used by the harness
    ins = reference.setup_inputs()
    out = kernel(**{k: np.asarray(x) for k, x in ins.items()})
    print(out.shape, out.dtype)
